# revision 47
# baseline (speedup 1.0000x reference)
"""Trainium2 Bass kernel: 2-layer GCN (GCNConv -> ReLU -> GCNConv -> Linear).

Strategy (8 NeuronCores, SPMD, 3 launches with host-side exchange):
  - Destination-node sharding with degree-balanced serpentine assignment.
  - NO on-device gathers: between launches the host pre-gathers the source
    rows of every edge into a dense per-core "stream" laid out in chunk
    order, so each launch only does large sequential DMA + PE matmuls.
      L1: H1 = X @ W1                      (row-sharded dense matmul)
      L2: MP1(H1-stream) + b1, ReLU, @ (W2@Wp) -> T2
      L3: MP2(T2-stream) + (b2@Wp + bp)    -> y (fp32)
  - Message passing: edges sorted by (dest tile, dest lane); chunks of 128
    edges contract with a narrow one-hot*norm weight window (lhsT) into the
    dest rows of a PSUM tile. Self-loops are ordinary edges in the stream.
  - All matmul operands bf16 (fp32 PSUM accumulation); final output fp32.
"""

import bisect
from contextlib import ExitStack
from dataclasses import dataclass, field

import numpy as np
import ml_dtypes

BF16 = ml_dtypes.bfloat16
FP32 = np.float32


# ---------------------------------------------------------------- config

@dataclass
class Cfg:
    N: int = 50000
    IN_DIM: int = 512
    HID: int = 256
    OUT: int = 128
    NCORES: int = 8

    ND: int = field(init=False)
    NTILES: int = field(init=False)
    NP: int = field(init=False)

    def __post_init__(self):
        self.ND = self.N // self.NCORES
        self.NTILES = (self.ND + 127) // 128
        self.NP = self.NTILES * 128


# ---------------------------------------------------------------- planner

class Plan:
    """Static (cross-core identical) geometry + per-core data arrays."""

    def __init__(self, cfg: Cfg, edge_index, edge_weight):
        self.cfg = cfg
        N, ND, NP, NT = cfg.N, cfg.ND, cfg.NP, cfg.NTILES
        NC = cfg.NCORES

        # --- gcn_norm with self loops; loops stay as ordinary edges
        row = np.concatenate([np.asarray(edge_index[0], np.int64),
                              np.arange(N, dtype=np.int64)])
        col = np.concatenate([np.asarray(edge_index[1], np.int64),
                              np.arange(N, dtype=np.int64)])
        w = np.concatenate([np.asarray(edge_weight, np.float64),
                            np.ones(N, np.float64)])
        deg = np.zeros(N, np.float64)
        np.add.at(deg, col, w)
        dinv = np.where(deg > 0, 1.0 / np.sqrt(deg), 0.0)
        nrm = (dinv[row] * w * dinv[col]).astype(np.float32)

        # --- degree-sorted serpentine node->(core, lane): every core gets a
        # near-identical per-tile edge-count profile -> minimal chunk padding
        degi = np.bincount(col, minlength=N)
        ranks = np.argsort(-degi, kind="stable")
        r = np.arange(N)
        blk = r // NC
        corepos = np.where(blk % 2 == 0, r % NC, NC - 1 - (r % NC))
        lane_global = np.empty(N, np.int64)        # node -> core*NP + lane
        lane_global[ranks] = corepos * NP + blk
        self.nodes = []                            # per core: lane -> node id
        for k in range(NC):
            nk = np.empty(ND, np.int64)
            sel = corepos == k
            nk[blk[sel]] = ranks[sel]
            self.nodes.append(nk)

        dst_core = lane_global[col] // NP
        dlane = lane_global[col] % NP
        dtile = dlane // 128
        dl = dlane % 128

        order = np.lexsort((dl, dtile, dst_core))
        so_core = dst_core[order]
        so_tile = dtile[order]
        so_lane = dl[order]
        so_src = lane_global[row[order]]           # table row of the source
        so_nrm = nrm[order]

        # chunks per (core, tile), padded to the cross-core max
        key = so_core * NT + so_tile
        cnt = np.bincount(key, minlength=NC * NT).reshape(NC, NT)
        self.CH = (-(-cnt // 128)).max(axis=0)     # [NT] static chunk counts
        self.cbase = np.concatenate([[0], np.cumsum(self.CH)])
        self.CTOT = int(self.cbase[-1])

        seg_start = np.concatenate(
            [[0], np.cumsum(np.bincount(key, minlength=NC * NT))])[:-1]
        rank = np.arange(len(key)) - seg_start[key]
        jglob = self.cbase[so_tile] + rank // 128  # global chunk index
        p = rank % 128                             # partition slot

        # final-layer stream sensitivity classes, one bit per STATIC chunk
        # (shared across cores): whole chunks carrying the lowest aggregate
        # norm^2 mass ride in fp8, bounded to ~25% of the total mass
        cmass = np.bincount(jglob, weights=nrm[order].astype(np.float64) ** 2,
                            minlength=self.CTOT)
        ordm = np.argsort(cmass)
        cum = np.cumsum(cmass[ordm])
        ncut = int(np.searchsorted(cum, 0.40 * cum[-1]))
        self.ccls = np.zeros(self.CTOT, np.int64)
        self.ccls[ordm[:ncut]] = 1
        self.f8_mass = float(cum[max(ncut - 1, 0)] / cum[-1])
        self.sidx = np.zeros(self.CTOT, np.int64)
        for c in (0, 1):
            m = self.ccls == c
            self.sidx[m] = np.arange(int(m.sum()))
        self.CTOTC = [int((self.ccls == 0).sum()), int((self.ccls == 1).sum())]

        # static output windows per global chunk (union over cores); the MP
        # matmul is transposed (dest lanes on the PSUM free dim), so windows
        # are exact [lo, hi] slices with no base-alignment constraint
        lo = np.full(self.CTOT, 128, np.int64)
        hi = np.full(self.CTOT, -1, np.int64)
        np.minimum.at(lo, jglob, so_lane)
        np.maximum.at(hi, jglob, so_lane)
        empty = hi < 0
        lo[empty], hi[empty] = 0, 0
        self.b0 = lo
        self.M = hi - lo + 1
        self.soff = np.concatenate([[0], np.cumsum(self.M)])
        self.SLAB = int(self.soff[-1])

        # --- per-core arrays
        self.srcpos = []   # global slot order [CTOT*128] (-1 = pad)
        self.srcposc = []  # per class: slots in class-stream order
        self.wslab = []    # [128, SLAB] bf16
        for k in range(NC):
            m = so_core == k
            sp = np.full(self.CTOT * 128, -1, np.int64)
            sp[jglob[m] * 128 + p[m]] = so_src[m]
            self.srcpos.append(sp)
            sp2 = sp.reshape(self.CTOT, 128)
            self.srcposc.append(
                [np.ascontiguousarray(sp2[self.ccls == c]).reshape(-1)
                 for c in (0, 1)])

            slab = np.zeros((128, self.SLAB), np.float32)
            slab[p[m], self.soff[jglob[m]] + so_lane[m] - self.b0[jglob[m]]] = \
                so_nrm[m]
            self.wslab.append(slab.astype(BF16))

    def build_stream(self, k, table, dtype=None):
        """Pre-gathered per-edge source rows, chunk-order layout [128, CTOT*F].

        table: [NC*NP, F]; slot (chunk j, partition p) -> columns j*F:(j+1)*F
        of SBUF partition p.  Padded slots read the appended zero row.
        """
        F = table.shape[1]
        if dtype is not None and table.dtype != dtype:
            table = table.astype(dtype)
        ext = np.vstack([table, np.zeros((1, F), table.dtype)])
        sp = self.srcpos[k].copy()
        sp[sp < 0] = table.shape[0]
        arr = ext[sp]
        return np.ascontiguousarray(
            arr.reshape(self.CTOT, 128, F).transpose(1, 0, 2)
            .reshape(128, self.CTOT * F))

    def build_stream_c(self, k, table, c, dtype=None):
        """Class-c subset of the stream, in class-stream chunk order."""
        F = table.shape[1]
        if dtype is not None and table.dtype != dtype:
            table = table.astype(dtype)
        ext = np.vstack([table, np.zeros((1, F), table.dtype)])
        sp = self.srcposc[k][c].copy()
        sp[sp < 0] = table.shape[0]
        n = self.CTOTC[c]
        arr = ext[sp]
        return np.ascontiguousarray(
            arr.reshape(n, 128, F).transpose(1, 0, 2).reshape(128, n * F))


# ---------------------------------------------------------------- bass builders

GB = 8           # tiles per DMA block (loads and output stores)


def _build_l1(cfg: Cfg):
    import concourse.bacc as bacc
    import concourse.mybir as mybir
    import concourse.tile as tile

    dt = mybir.dt
    nc = bacc.Bacc(None, target_bir_lowering=False)
    KCH = cfg.IN_DIM // 128
    NT = cfg.NTILES
    NB = -(-NT // GB)
    xt = nc.dram_tensor("xt", [128, NT * cfg.IN_DIM], dt.bfloat16,
                        kind="ExternalInput")
    w1 = nc.dram_tensor("w1", [128, KCH * cfg.HID], dt.bfloat16,
                        kind="ExternalInput")
    # partition-major: h1[p, t*HID:(t+1)*HID] = row (t*128+p) of the shard
    h1 = nc.dram_tensor("h1", [128, NT * cfg.HID], dt.bfloat16,
                        kind="ExternalOutput")

    with tile.TileContext(nc) as tc, ExitStack() as ctx:
        consts = ctx.enter_context(tc.tile_pool(name="consts", bufs=1))
        xts = ctx.enter_context(tc.tile_pool(name="xts", bufs=7))
        outs = ctx.enter_context(tc.tile_pool(name="outs", bufs=3))
        psum = ctx.enter_context(tc.tile_pool(name="psum", bufs=6, space="PSUM"))

        w1_sb = consts.tile([128, KCH * cfg.HID], dt.bfloat16, tag="w1")
        nc.scalar.dma_start(w1_sb[:], w1[:])

        xblocks = []
        for b in range(NB):
            nt = min(GB, NT - b * GB)
            xb = xts.tile([128, GB * cfg.IN_DIM], dt.bfloat16, tag="xt")
            eng = nc.scalar if b == 0 else nc.sync   # first block in parallel
            eng.dma_start(
                xb[:, :nt * cfg.IN_DIM],
                xt[:, b * GB * cfg.IN_DIM:(b * GB + nt) * cfg.IN_DIM])
            xblocks.append(xb)

        ostage = None
        for t in range(NT):
            if t % GB == 0:
                ostage = outs.tile([128, GB * cfg.HID], dt.bfloat16, tag="o")
            xb = xblocks[t // GB]
            xoff = (t % GB) * cfg.IN_DIM
            ps = psum.tile([128, cfg.HID], dt.float32)
            for c in range(KCH):
                nc.tensor.matmul(
                    ps[:],
                    xb[:, xoff + c * 128:xoff + (c + 1) * 128],
                    w1_sb[:, c * cfg.HID:(c + 1) * cfg.HID],
                    start=(c == 0), stop=(c == KCH - 1),
                )
            so = (t % GB) * cfg.HID
            nc.scalar.activation(ostage[:, so:so + cfg.HID], ps[:],
                                 mybir.ActivationFunctionType.Copy)
            if t % GB == GB - 1 or t == NT - 1:
                g0 = (t // GB) * GB
                nt = t - g0 + 1
                nc.scalar.dma_start(
                    h1[:, g0 * cfg.HID:(g0 + nt) * cfg.HID],
                    ostage[:, :nt * cfg.HID])
    nc.finalize()
    return nc


FP8_L2_STREAM = True     # halve the dominant DMA stream (host-sim ~1.54e-2)


def _build_mp(cfg: Cfg, plan: Plan, layer2: bool):
    """Transposed message passing: psT[feat, dest] += g_chunk.T @ wsl_chunk.

    The gathered edge-feature chunk is the stationary operand and the one-hot
    weight window the moving one, so dest-lane windows live on the PSUM free
    dim (no base alignment), the per-feature bias becomes a per-partition
    activation bias, and the @W2p matmul consumes psT directly (no PE
    transposes).  Output is T2^T / y^T, un-transposed by the host for free.

    layer2: relu(MP1 + b1) @ W2p -> T2^T (bf16).
    else:   MP2 + bpp            -> y^T (bf16)."""
    import concourse.bacc as bacc
    import concourse.mybir as mybir
    import concourse.tile as tile

    dt = mybir.dt
    F = cfg.HID if layer2 else cfg.OUT
    FCH = F // 128
    nc = bacc.Bacc(None, target_bir_lowering=False)

    if layer2:
        sdt = dt.float8e4 if FP8_L2_STREAM else dt.bfloat16
        stream = nc.dram_tensor("stream", [128, plan.CTOT * F], sdt,
                                kind="ExternalInput")
    else:
        # low-sensitivity edge class rides in fp8
        stream0 = nc.dram_tensor("stream0", [128, plan.CTOTC[0] * F],
                                 dt.bfloat16, kind="ExternalInput")
        stream1 = nc.dram_tensor("stream1", [128, plan.CTOTC[1] * F],
                                 dt.float8e4, kind="ExternalInput")
    wsl = nc.dram_tensor("wsl", [128, plan.SLAB], dt.bfloat16,
                         kind="ExternalInput")
    bias = nc.dram_tensor("bias", [128, FCH], dt.float32,
                          kind="ExternalInput")
    if layer2:
        wnext = nc.dram_tensor("wnext", [128, FCH * cfg.OUT], dt.bfloat16,
                               kind="ExternalInput")
    # per tile t, columns [t*128, (t+1)*128) hold the TRANSPOSED result
    # ([feature, dest lane]); the host un-transposes
    out = nc.dram_tensor("out", [128, cfg.NTILES * 128], dt.bfloat16,
                         kind="ExternalOutput")

    BS = 48          # stream chunks per DMA block
    NB = -(-plan.CTOT // BS)
    NT = cfg.NTILES

    with tile.TileContext(nc) as tc, ExitStack() as ctx:
        consts = ctx.enter_context(tc.tile_pool(name="consts", bufs=1))
        gpool = ctx.enter_context(
            tc.tile_pool(name="gpool", bufs=7 if layer2 else 6))
        work = ctx.enter_context(tc.tile_pool(name="work", bufs=4))
        outs = ctx.enter_context(tc.tile_pool(name="outs", bufs=3))
        psmp = ctx.enter_context(
            tc.tile_pool(name="psmp", bufs=4 if layer2 else 6, space="PSUM"))
        if layer2:
            psmm = ctx.enter_context(
                tc.tile_pool(name="psmm", bufs=2, space="PSUM"))

        bias_sb = consts.tile([128, FCH], dt.float32, tag="bias")
        nc.scalar.dma_start(bias_sb[:], bias[:])
        if layer2:
            wnext_sb = consts.tile([128, FCH * cfg.OUT], dt.bfloat16,
                                   tag="wnext")
            nc.scalar.dma_start(wnext_sb[:], wnext[:])

        # weight slab in a few big pieces on the ACT queue (parallel to the
        # stream queue; per-chunk pieces made thousands of 520B descriptors)
        NWP = 3
        jb = [i * plan.CTOT // NWP for i in range(NWP)] + [plan.CTOT]
        wblocks, wjb = [], []
        for i in range(NWP):
            w0 = int(plan.soff[jb[i]])
            w1_ = int(plan.soff[jb[i + 1]])
            wb = consts.tile([128, max(w1_ - w0, 1)], dt.bfloat16,
                             tag=f"w{i}")
            nc.scalar.dma_start(wb[:], wsl[:, w0:w1_])
            wblocks.append(wb)
            wjb.append(jb[i])

        def emit_gblock(dram_t, n_chunks, b, gdt, tagp):
            nchk = min(BS, n_chunks - b * BS)
            gb = gpool.tile([128, BS * F], gdt, tag=tagp)
            nc.sync.dma_start(gb[:, :nchk * F],
                              dram_t[:, b * BS * F:(b * BS + nchk) * F])
            return gb

        if layer2:
            gblocks = [emit_gblock(stream, plan.CTOT, b, sdt, "g")
                       for b in range(-(-plan.CTOT // BS))]
        else:
            # interleave the two class streams in consumption proportion
            n0 = -(-plan.CTOTC[0] // BS)
            n1 = -(-plan.CTOTC[1] // BS)
            gb0, gb1 = [], []
            i0 = i1 = 0
            while i0 < n0 or i1 < n1:
                if i1 >= n1 or (i0 < n0 and i0 * (n1 + 1) <= i1 * (n0 + 1)):
                    gb0.append(emit_gblock(stream0, plan.CTOTC[0], i0,
                                           dt.bfloat16, "g0"))
                    i0 += 1
                else:
                    gb1.append(emit_gblock(stream1, plan.CTOTC[1], i1,
                                           dt.float8e4, "g1"))
                    i1 += 1

        state = {}
        ostage = [None]

        def stage_mp(t):
            chunks = list(range(int(plan.cbase[t]),
                                int(plan.cbase[t]) + int(plan.CH[t])))
            ps = psmp.tile([128, FCH * 128], dt.float32)
            nc.vector.memset(ps[:], 0.0)
            for ci, j in enumerate(chunks):
                b0 = int(plan.b0[j])
                M = int(plan.M[j])
                wp = bisect.bisect_right(wjb, j) - 1
                so = int(plan.soff[j]) - int(plan.soff[wjb[wp]])
                if layer2:
                    gb = gblocks[j // BS]
                    goff = (j % BS) * F
                else:
                    si = int(plan.sidx[j])
                    gb = (gb0, gb1)[int(plan.ccls[j])][si // BS]
                    goff = (si % BS) * F
                last = ci == len(chunks) - 1
                for fc in range(FCH):
                    nc.tensor.matmul(
                        ps[:, fc * 128 + b0:fc * 128 + b0 + M],
                        gb[:, goff + fc * 128:goff + (fc + 1) * 128],
                        wblocks[wp][:, so:so + M],
                        start=False, stop=last and fc == FCH - 1,
                        skip_group_check=True,
                    )
            state[t] = ps

        def stage_out(t):
            if t % GB == 0:
                o_t = outs.tile([128, GB * 128], dt.bfloat16, tag="o")
                ostage[0] = o_t
            so_ = (t % GB) * 128
            ps = state.pop(t)
            if layer2:
                actT = work.tile([128, F], dt.bfloat16, tag="act")
                for fc in range(FCH):
                    nc.scalar.activation(
                        actT[:, fc * 128:(fc + 1) * 128],
                        ps[:, fc * 128:(fc + 1) * 128],
                        mybir.ActivationFunctionType.Relu,
                        bias=bias_sb[:, fc:fc + 1])
                ps2 = psmm.tile([128, cfg.OUT], dt.float32)
                for fc in range(FCH):
                    nc.tensor.matmul(
                        ps2[:],
                        wnext_sb[:, fc * cfg.OUT:(fc + 1) * cfg.OUT],
                        actT[:, fc * 128:(fc + 1) * 128],
                        start=(fc == 0), stop=(fc == FCH - 1))
                nc.scalar.activation(ostage[0][:, so_:so_ + 128], ps2[:],
                                     mybir.ActivationFunctionType.Copy)
            else:
                nc.scalar.activation(ostage[0][:, so_:so_ + 128], ps[:],
                                     mybir.ActivationFunctionType.Identity,
                                     bias=bias_sb[:, 0:1])
            if t % GB == GB - 1 or t == NT - 1:
                g0 = (t // GB) * GB
                nt = t - g0 + 1
                nc.scalar.dma_start(
                    out[:, g0 * 128:(g0 + nt) * 128],
                    ostage[0][:, :nt * 128])

        lag = 2 if layer2 else 1
        for u in range(NT + lag):
            if u < NT:
                stage_mp(u)
            if 0 <= u - lag < NT:
                stage_out(u - lag)

    nc.finalize()
    return nc


# ---------------------------------------------------------------- host packing

def _pack_l1_inputs(cfg: Cfg, plan: Plan, x, W1):
    KCH = cfg.IN_DIM // 128
    w1r = np.zeros((128, KCH * cfg.HID), BF16)
    for c in range(KCH):
        w1r[:, c * cfg.HID:(c + 1) * cfg.HID] = \
            W1[c * 128:(c + 1) * 128, :].astype(BF16)
    maps = []
    for k in range(cfg.NCORES):
        xs = np.zeros((cfg.NP, cfg.IN_DIM), np.float32)
        xs[:cfg.ND] = x[plan.nodes[k]]
        # xt[p, t*IN + c*128 + q] = xs[t*128 + q, c*128 + p]
        xtr = np.ascontiguousarray(
            xs.reshape(cfg.NTILES, 128, KCH, 128).transpose(3, 0, 2, 1)
            .reshape(128, cfg.NTILES * cfg.IN_DIM)).astype(BF16)
        maps.append({"xt": xtr, "w1": w1r})
    return maps


def _pack_mp_inputs(cfg: Cfg, plan: Plan, table, Wn, b, layer2):
    F = cfg.HID if layer2 else cfg.OUT
    FCH = F // 128
    # per-partition bias columns: bias[p, fc] = b[fc*128 + p]
    biasr = np.ascontiguousarray(
        b.astype(np.float32).reshape(FCH, 128).T)
    maps = []
    for k in range(cfg.NCORES):
        if layer2:
            sdt = ml_dtypes.float8_e4m3 if FP8_L2_STREAM else None
            m = {"stream": plan.build_stream(k, table, dtype=sdt)}
        else:
            m = {"stream0": plan.build_stream_c(k, table, 0),
                 "stream1": plan.build_stream_c(
                     k, table, 1, dtype=ml_dtypes.float8_e4m3)}
        m["wsl"] = plan.wslab[k]
        m["bias"] = biasr
        if layer2:
            wnr = np.zeros((128, FCH * cfg.OUT), BF16)
            for c in range(FCH):
                wnr[:, c * cfg.OUT:(c + 1) * cfg.OUT] = \
                    Wn[c * 128:(c + 1) * 128, :].astype(BF16)
            m["wnext"] = wnr
        maps.append(m)
    return maps


# ---------------------------------------------------------------- driver

def kernel_run(inputs, cfg=None, trace=False):
    from concourse.bass_utils import run_bass_kernel_spmd

    cfg = cfg or Cfg()
    x = np.asarray(inputs["x"], np.float32)
    plan = Plan(cfg, np.asarray(inputs["edge_index"]),
                np.asarray(inputs["edge_weight"], np.float32))
    W1 = np.asarray(inputs["W1"], np.float32)
    b1 = np.asarray(inputs["b1"], np.float32)
    W2 = np.asarray(inputs["W2"], np.float32)
    b2 = np.asarray(inputs["b2"], np.float32)
    Wp = np.asarray(inputs["Wp"], np.float32)
    bp = np.asarray(inputs["bp"], np.float32)

    results = []

    def run(build, maps, outname):
        nc = build()
        r = run_bass_kernel_spmd(nc, maps, list(range(cfg.NCORES)),
                                 trace=trace)
        results.append(r)
        return r.results

    def as_bf16(a):
        a = np.asarray(a)
        return a if a.dtype == BF16 else a.view(BF16)

    def unpack(a, F):
        # [128, NT*F] partition-major -> [NP, F] row-major
        return np.ascontiguousarray(
            a.reshape(128, cfg.NTILES, F).transpose(1, 0, 2)
            .reshape(cfg.NP, F))

    def unpack_T(a, F):
        # [F, NT*128] transposed tiles -> [NP, F] row-major
        return np.ascontiguousarray(
            a.reshape(F, cfg.NTILES, 128).transpose(1, 2, 0)
            .reshape(cfg.NP, F))

    # fold the post-projection into layer 2: A(relu1@W2)@Wp = A(relu1@(W2@Wp))
    W2p = (W2 @ Wp).astype(np.float32)
    bpp = (b2 @ Wp + bp).astype(np.float32)

    r1 = run(lambda: _build_l1(cfg), _pack_l1_inputs(cfg, plan, x, W1), "h1")
    T1 = np.concatenate([unpack(as_bf16(r["h1"]), cfg.HID) for r in r1],
                        axis=0)

    r2 = run(lambda: _build_mp(cfg, plan, True),
             _pack_mp_inputs(cfg, plan, T1, W2p, b1, True), "out")
    T2 = np.concatenate([unpack_T(as_bf16(r["out"]), cfg.OUT) for r in r2],
                        axis=0)

    r3 = run(lambda: _build_mp(cfg, plan, False),
             _pack_mp_inputs(cfg, plan, T2, None, bpp, False), "out")

    y = np.empty((cfg.N, cfg.OUT), np.float32)
    for k in range(cfg.NCORES):
        shard = unpack_T(as_bf16(r3[k]["out"]), cfg.OUT).astype(np.float32)
        y[plan.nodes[k]] = shard[:cfg.ND]
    return y, results


def kernel(**inputs):
    y, _ = kernel_run(inputs)
    return y


# revision 50
# speedup vs baseline: 1.0291x; 1.0291x over previous
"""Trainium2 Bass kernel: 2-layer GCN (GCNConv -> ReLU -> GCNConv -> Linear).

Strategy (8 NeuronCores, SPMD, 3 launches with host-side exchange):
  - Destination-node sharding with degree-balanced serpentine assignment.
  - NO on-device gathers: between launches the host pre-gathers the source
    rows of every edge into a dense per-core "stream" laid out in chunk
    order, so each launch only does large sequential DMA + PE matmuls.
      L1: H1 = X @ W1                      (row-sharded dense matmul)
      L2: MP1(H1-stream) + b1, ReLU, @ (W2@Wp) -> T2
      L3: MP2(T2-stream) + (b2@Wp + bp)    -> y (fp32)
  - Message passing: edges sorted by (dest tile, dest lane); chunks of 128
    edges contract with a narrow one-hot*norm weight window (lhsT) into the
    dest rows of a PSUM tile. Self-loops are ordinary edges in the stream.
  - All matmul operands bf16 (fp32 PSUM accumulation); final output fp32.
"""

import bisect
from contextlib import ExitStack
from dataclasses import dataclass, field

import numpy as np
import ml_dtypes

BF16 = ml_dtypes.bfloat16
FP32 = np.float32


# ---------------------------------------------------------------- config

@dataclass
class Cfg:
    N: int = 50000
    IN_DIM: int = 512
    HID: int = 256
    OUT: int = 128
    NCORES: int = 8

    ND: int = field(init=False)
    NTILES: int = field(init=False)
    NP: int = field(init=False)

    def __post_init__(self):
        self.ND = self.N // self.NCORES
        self.NTILES = (self.ND + 127) // 128
        self.NP = self.NTILES * 128


# ---------------------------------------------------------------- planner

class Plan:
    """Static (cross-core identical) geometry + per-core data arrays."""

    def __init__(self, cfg: Cfg, edge_index, edge_weight):
        self.cfg = cfg
        N, ND, NP, NT = cfg.N, cfg.ND, cfg.NP, cfg.NTILES
        NC = cfg.NCORES

        # --- gcn_norm with self loops; loops stay as ordinary edges
        row = np.concatenate([np.asarray(edge_index[0], np.int64),
                              np.arange(N, dtype=np.int64)])
        col = np.concatenate([np.asarray(edge_index[1], np.int64),
                              np.arange(N, dtype=np.int64)])
        w = np.concatenate([np.asarray(edge_weight, np.float64),
                            np.ones(N, np.float64)])
        deg = np.zeros(N, np.float64)
        np.add.at(deg, col, w)
        dinv = np.where(deg > 0, 1.0 / np.sqrt(deg), 0.0)
        nrm = (dinv[row] * w * dinv[col]).astype(np.float32)

        # --- degree-sorted serpentine node->(core, lane): every core gets a
        # near-identical per-tile edge-count profile -> minimal chunk padding
        degi = np.bincount(col, minlength=N)
        ranks = np.argsort(-degi, kind="stable")
        r = np.arange(N)
        blk = r // NC
        corepos = np.where(blk % 2 == 0, r % NC, NC - 1 - (r % NC))
        lane_global = np.empty(N, np.int64)        # node -> core*NP + lane
        lane_global[ranks] = corepos * NP + blk
        self.nodes = []                            # per core: lane -> node id
        for k in range(NC):
            nk = np.empty(ND, np.int64)
            sel = corepos == k
            nk[blk[sel]] = ranks[sel]
            self.nodes.append(nk)

        dst_core = lane_global[col] // NP
        dlane = lane_global[col] % NP
        dtile = dlane // 128
        dl = dlane % 128

        order = np.lexsort((dl, dtile, dst_core))
        so_core = dst_core[order]
        so_tile = dtile[order]
        so_lane = dl[order]
        so_src = lane_global[row[order]]           # table row of the source
        so_nrm = nrm[order]

        # chunks per (core, tile), padded to the cross-core max
        key = so_core * NT + so_tile
        cnt = np.bincount(key, minlength=NC * NT).reshape(NC, NT)
        self.CH = (-(-cnt // 128)).max(axis=0)     # [NT] static chunk counts
        self.cbase = np.concatenate([[0], np.cumsum(self.CH)])
        self.CTOT = int(self.cbase[-1])

        seg_start = np.concatenate(
            [[0], np.cumsum(np.bincount(key, minlength=NC * NT))])[:-1]
        rank = np.arange(len(key)) - seg_start[key]
        jglob = self.cbase[so_tile] + rank // 128  # global chunk index
        p = rank % 128                             # partition slot

        # final-layer stream sensitivity classes, one bit per STATIC chunk
        # (shared across cores): whole chunks carrying the lowest aggregate
        # norm^2 mass ride in fp8, bounded to ~25% of the total mass
        cmass = np.bincount(jglob, weights=nrm[order].astype(np.float64) ** 2,
                            minlength=self.CTOT)
        ordm = np.argsort(cmass)
        cum = np.cumsum(cmass[ordm])
        ncut = int(np.searchsorted(cum, 0.40 * cum[-1]))
        self.ccls = np.zeros(self.CTOT, np.int64)
        self.ccls[ordm[:ncut]] = 1
        self.f8_mass = float(cum[max(ncut - 1, 0)] / cum[-1])
        self.sidx = np.zeros(self.CTOT, np.int64)
        for c in (0, 1):
            m = self.ccls == c
            self.sidx[m] = np.arange(int(m.sum()))
        self.CTOTC = [int((self.ccls == 0).sum()), int((self.ccls == 1).sum())]

        # static output windows per global chunk (union over cores); the MP
        # matmul is transposed (dest lanes on the PSUM free dim), so windows
        # are exact [lo, hi] slices with no base-alignment constraint
        lo = np.full(self.CTOT, 128, np.int64)
        hi = np.full(self.CTOT, -1, np.int64)
        np.minimum.at(lo, jglob, so_lane)
        np.maximum.at(hi, jglob, so_lane)
        empty = hi < 0
        lo[empty], hi[empty] = 0, 0
        self.b0 = lo
        self.M = hi - lo + 1
        self.soff = np.concatenate([[0], np.cumsum(self.M)])
        self.SLAB = int(self.soff[-1])

        # --- per-core arrays
        self.srcpos = []   # global slot order [CTOT*128] (-1 = pad)
        self.srcposc = []  # per class: slots in class-stream order
        self.wslab = []    # [128, SLAB] bf16
        for k in range(NC):
            m = so_core == k
            sp = np.full(self.CTOT * 128, -1, np.int64)
            sp[jglob[m] * 128 + p[m]] = so_src[m]
            self.srcpos.append(sp)
            sp2 = sp.reshape(self.CTOT, 128)
            self.srcposc.append(
                [np.ascontiguousarray(sp2[self.ccls == c]).reshape(-1)
                 for c in (0, 1)])

            slab = np.zeros((128, self.SLAB), np.float32)
            slab[p[m], self.soff[jglob[m]] + so_lane[m] - self.b0[jglob[m]]] = \
                so_nrm[m]
            self.wslab.append(slab.astype(BF16))

    def build_stream(self, k, table, dtype=None):
        """Pre-gathered per-edge source rows, chunk-order layout [128, CTOT*F].

        table: [NC*NP, F]; slot (chunk j, partition p) -> columns j*F:(j+1)*F
        of SBUF partition p.  Padded slots read the appended zero row.
        """
        F = table.shape[1]
        if dtype is not None and table.dtype != dtype:
            table = table.astype(dtype)
        ext = np.vstack([table, np.zeros((1, F), table.dtype)])
        sp = self.srcpos[k].copy()
        sp[sp < 0] = table.shape[0]
        arr = ext[sp]
        return np.ascontiguousarray(
            arr.reshape(self.CTOT, 128, F).transpose(1, 0, 2)
            .reshape(128, self.CTOT * F))

    def build_stream_c(self, k, table, c, dtype=None):
        """Class-c subset of the stream, in class-stream chunk order."""
        F = table.shape[1]
        if dtype is not None and table.dtype != dtype:
            table = table.astype(dtype)
        ext = np.vstack([table, np.zeros((1, F), table.dtype)])
        sp = self.srcposc[k][c].copy()
        sp[sp < 0] = table.shape[0]
        n = self.CTOTC[c]
        arr = ext[sp]
        return np.ascontiguousarray(
            arr.reshape(n, 128, F).transpose(1, 0, 2).reshape(128, n * F))


# ---------------------------------------------------------------- bass builders

GB = 8           # tiles per DMA block (loads and output stores)


def _build_l1(cfg: Cfg):
    import concourse.bacc as bacc
    import concourse.mybir as mybir
    import concourse.tile as tile

    dt = mybir.dt
    nc = bacc.Bacc(None, target_bir_lowering=False)
    KCH = cfg.IN_DIM // 128
    NT = cfg.NTILES
    NB = -(-NT // GB)
    xt = nc.dram_tensor("xt", [128, NT * cfg.IN_DIM], dt.bfloat16,
                        kind="ExternalInput")
    w1 = nc.dram_tensor("w1", [128, KCH * cfg.HID], dt.bfloat16,
                        kind="ExternalInput")
    # partition-major: h1[p, t*HID:(t+1)*HID] = row (t*128+p) of the shard
    h1 = nc.dram_tensor("h1", [128, NT * cfg.HID], dt.bfloat16,
                        kind="ExternalOutput")

    with tile.TileContext(nc) as tc, ExitStack() as ctx:
        consts = ctx.enter_context(tc.tile_pool(name="consts", bufs=1))
        xts = ctx.enter_context(tc.tile_pool(name="xts", bufs=7))
        outs = ctx.enter_context(tc.tile_pool(name="outs", bufs=3))
        psum = ctx.enter_context(tc.tile_pool(name="psum", bufs=6, space="PSUM"))

        w1_sb = consts.tile([128, KCH * cfg.HID], dt.bfloat16, tag="w1")
        nc.scalar.dma_start(w1_sb[:], w1[:])

        xblocks = []
        for b in range(NB):
            nt = min(GB, NT - b * GB)
            xb = xts.tile([128, GB * cfg.IN_DIM], dt.bfloat16, tag="xt")
            eng = nc.sync if b % 2 == 0 else nc.scalar
            eng.dma_start(
                xb[:, :nt * cfg.IN_DIM],
                xt[:, b * GB * cfg.IN_DIM:(b * GB + nt) * cfg.IN_DIM])
            xblocks.append(xb)

        ostage = None
        for t in range(NT):
            if t % GB == 0:
                ostage = outs.tile([128, GB * cfg.HID], dt.bfloat16, tag="o")
            xb = xblocks[t // GB]
            xoff = (t % GB) * cfg.IN_DIM
            ps = psum.tile([128, cfg.HID], dt.float32)
            for c in range(KCH):
                nc.tensor.matmul(
                    ps[:],
                    xb[:, xoff + c * 128:xoff + (c + 1) * 128],
                    w1_sb[:, c * cfg.HID:(c + 1) * cfg.HID],
                    start=(c == 0), stop=(c == KCH - 1),
                )
            so = (t % GB) * cfg.HID
            nc.scalar.activation(ostage[:, so:so + cfg.HID], ps[:],
                                 mybir.ActivationFunctionType.Copy)
            if t % GB == GB - 1 or t == NT - 1:
                g0 = (t // GB) * GB
                nt = t - g0 + 1
                nc.scalar.dma_start(
                    h1[:, g0 * cfg.HID:(g0 + nt) * cfg.HID],
                    ostage[:, :nt * cfg.HID])
    nc.finalize()
    return nc


FP8_L2_STREAM = True     # halve the dominant DMA stream (host-sim ~1.54e-2)


def _build_mp(cfg: Cfg, plan: Plan, layer2: bool):
    """Transposed message passing: psT[feat, dest] += g_chunk.T @ wsl_chunk.

    The gathered edge-feature chunk is the stationary operand and the one-hot
    weight window the moving one, so dest-lane windows live on the PSUM free
    dim (no base alignment), the per-feature bias becomes a per-partition
    activation bias, and the @W2p matmul consumes psT directly (no PE
    transposes).  Output is T2^T / y^T, un-transposed by the host for free.

    layer2: relu(MP1 + b1) @ W2p -> T2^T (bf16).
    else:   MP2 + bpp            -> y^T (bf16)."""
    import concourse.bacc as bacc
    import concourse.mybir as mybir
    import concourse.tile as tile

    dt = mybir.dt
    F = cfg.HID if layer2 else cfg.OUT
    FCH = F // 128
    nc = bacc.Bacc(None, target_bir_lowering=False)

    if layer2:
        sdt = dt.float8e4 if FP8_L2_STREAM else dt.bfloat16
        stream = nc.dram_tensor("stream", [128, plan.CTOT * F], sdt,
                                kind="ExternalInput")
    else:
        # low-sensitivity edge class rides in fp8
        stream0 = nc.dram_tensor("stream0", [128, plan.CTOTC[0] * F],
                                 dt.bfloat16, kind="ExternalInput")
        stream1 = nc.dram_tensor("stream1", [128, plan.CTOTC[1] * F],
                                 dt.float8e4, kind="ExternalInput")
    wsl = nc.dram_tensor("wsl", [128, plan.SLAB], dt.bfloat16,
                         kind="ExternalInput")
    bias = nc.dram_tensor("bias", [128, FCH], dt.float32,
                          kind="ExternalInput")
    if layer2:
        wnext = nc.dram_tensor("wnext", [128, FCH * cfg.OUT], dt.bfloat16,
                               kind="ExternalInput")
    # per tile t, columns [t*128, (t+1)*128) hold the TRANSPOSED result
    # ([feature, dest lane]); the host un-transposes
    out = nc.dram_tensor("out", [128, cfg.NTILES * 128], dt.bfloat16,
                         kind="ExternalOutput")

    BS = 32          # stream chunks per DMA block
    NB = -(-plan.CTOT // BS)
    NT = cfg.NTILES

    with tile.TileContext(nc) as tc, ExitStack() as ctx:
        consts = ctx.enter_context(tc.tile_pool(name="consts", bufs=1))
        gpool = ctx.enter_context(tc.tile_pool(name="gpool", bufs=10))
        work = ctx.enter_context(tc.tile_pool(name="work", bufs=4))
        outs = ctx.enter_context(tc.tile_pool(name="outs", bufs=3))
        psmp = ctx.enter_context(
            tc.tile_pool(name="psmp", bufs=4 if layer2 else 6, space="PSUM"))
        if layer2:
            psmm = ctx.enter_context(
                tc.tile_pool(name="psmm", bufs=2, space="PSUM"))

        bias_sb = consts.tile([128, FCH], dt.float32, tag="bias")
        nc.scalar.dma_start(bias_sb[:], bias[:])
        if layer2:
            wnext_sb = consts.tile([128, FCH * cfg.OUT], dt.bfloat16,
                                   tag="wnext")
            nc.scalar.dma_start(wnext_sb[:], wnext[:])

        # weight slab in a few big pieces on the ACT queue (parallel to the
        # stream queue; per-chunk pieces made thousands of 520B descriptors)
        NWP = 3
        jb = [i * plan.CTOT // NWP for i in range(NWP)] + [plan.CTOT]
        wblocks, wjb = [], []
        for i in range(NWP):
            w0 = int(plan.soff[jb[i]])
            w1_ = int(plan.soff[jb[i + 1]])
            wb = consts.tile([128, max(w1_ - w0, 1)], dt.bfloat16,
                             tag=f"w{i}")
            nc.scalar.dma_start(wb[:], wsl[:, w0:w1_])
            wblocks.append(wb)
            wjb.append(jb[i])

        # alternate stream blocks across both HWDGE queues: one queue's
        # descriptor dispatch (~35ns/desc) can't keep 16 DMA engines fed
        gq = [0]

        def emit_gblock(dram_t, n_chunks, b, gdt, tagp):
            nchk = min(BS, n_chunks - b * BS)
            gb = gpool.tile([128, BS * F], gdt, tag=tagp)
            eng = nc.sync if gq[0] % 2 == 0 else nc.scalar
            gq[0] += 1
            eng.dma_start(gb[:, :nchk * F],
                          dram_t[:, b * BS * F:(b * BS + nchk) * F])
            return gb

        if layer2:
            gblocks = [emit_gblock(stream, plan.CTOT, b, sdt, "g")
                       for b in range(-(-plan.CTOT // BS))]
        else:
            # interleave the two class streams in consumption proportion
            n0 = -(-plan.CTOTC[0] // BS)
            n1 = -(-plan.CTOTC[1] // BS)
            gb0, gb1 = [], []
            i0 = i1 = 0
            while i0 < n0 or i1 < n1:
                if i1 >= n1 or (i0 < n0 and i0 * (n1 + 1) <= i1 * (n0 + 1)):
                    gb0.append(emit_gblock(stream0, plan.CTOTC[0], i0,
                                           dt.bfloat16, "g0"))
                    i0 += 1
                else:
                    gb1.append(emit_gblock(stream1, plan.CTOTC[1], i1,
                                           dt.float8e4, "g1"))
                    i1 += 1

        state = {}
        ostage = [None]

        def stage_mp(t):
            chunks = list(range(int(plan.cbase[t]),
                                int(plan.cbase[t]) + int(plan.CH[t])))
            ps = psmp.tile([128, FCH * 128], dt.float32)
            nc.vector.memset(ps[:], 0.0)
            for ci, j in enumerate(chunks):
                b0 = int(plan.b0[j])
                M = int(plan.M[j])
                wp = bisect.bisect_right(wjb, j) - 1
                so = int(plan.soff[j]) - int(plan.soff[wjb[wp]])
                if layer2:
                    gb = gblocks[j // BS]
                    goff = (j % BS) * F
                else:
                    si = int(plan.sidx[j])
                    gb = (gb0, gb1)[int(plan.ccls[j])][si // BS]
                    goff = (si % BS) * F
                last = ci == len(chunks) - 1
                for fc in range(FCH):
                    nc.tensor.matmul(
                        ps[:, fc * 128 + b0:fc * 128 + b0 + M],
                        gb[:, goff + fc * 128:goff + (fc + 1) * 128],
                        wblocks[wp][:, so:so + M],
                        start=False, stop=last and fc == FCH - 1,
                        skip_group_check=True,
                    )
            state[t] = ps

        def stage_out(t):
            if t % GB == 0:
                o_t = outs.tile([128, GB * 128], dt.bfloat16, tag="o")
                ostage[0] = o_t
            so_ = (t % GB) * 128
            ps = state.pop(t)
            if layer2:
                actT = work.tile([128, F], dt.bfloat16, tag="act")
                for fc in range(FCH):
                    nc.scalar.activation(
                        actT[:, fc * 128:(fc + 1) * 128],
                        ps[:, fc * 128:(fc + 1) * 128],
                        mybir.ActivationFunctionType.Relu,
                        bias=bias_sb[:, fc:fc + 1])
                ps2 = psmm.tile([128, cfg.OUT], dt.float32)
                for fc in range(FCH):
                    nc.tensor.matmul(
                        ps2[:],
                        wnext_sb[:, fc * cfg.OUT:(fc + 1) * cfg.OUT],
                        actT[:, fc * 128:(fc + 1) * 128],
                        start=(fc == 0), stop=(fc == FCH - 1))
                nc.scalar.activation(ostage[0][:, so_:so_ + 128], ps2[:],
                                     mybir.ActivationFunctionType.Copy)
            else:
                nc.scalar.activation(ostage[0][:, so_:so_ + 128], ps[:],
                                     mybir.ActivationFunctionType.Identity,
                                     bias=bias_sb[:, 0:1])
            if t % GB == GB - 1 or t == NT - 1:
                g0 = (t // GB) * GB
                nt = t - g0 + 1
                nc.scalar.dma_start(
                    out[:, g0 * 128:(g0 + nt) * 128],
                    ostage[0][:, :nt * 128])

        lag = 2 if layer2 else 1
        for u in range(NT + lag):
            if u < NT:
                stage_mp(u)
            if 0 <= u - lag < NT:
                stage_out(u - lag)

    nc.finalize()
    return nc


# ---------------------------------------------------------------- host packing

def _pack_l1_inputs(cfg: Cfg, plan: Plan, x, W1):
    KCH = cfg.IN_DIM // 128
    w1r = np.zeros((128, KCH * cfg.HID), BF16)
    for c in range(KCH):
        w1r[:, c * cfg.HID:(c + 1) * cfg.HID] = \
            W1[c * 128:(c + 1) * 128, :].astype(BF16)
    maps = []
    for k in range(cfg.NCORES):
        xs = np.zeros((cfg.NP, cfg.IN_DIM), np.float32)
        xs[:cfg.ND] = x[plan.nodes[k]]
        # xt[p, t*IN + c*128 + q] = xs[t*128 + q, c*128 + p]
        xtr = np.ascontiguousarray(
            xs.reshape(cfg.NTILES, 128, KCH, 128).transpose(3, 0, 2, 1)
            .reshape(128, cfg.NTILES * cfg.IN_DIM)).astype(BF16)
        maps.append({"xt": xtr, "w1": w1r})
    return maps


def _pack_mp_inputs(cfg: Cfg, plan: Plan, table, Wn, b, layer2):
    F = cfg.HID if layer2 else cfg.OUT
    FCH = F // 128
    # per-partition bias columns: bias[p, fc] = b[fc*128 + p]
    biasr = np.ascontiguousarray(
        b.astype(np.float32).reshape(FCH, 128).T)
    maps = []
    for k in range(cfg.NCORES):
        if layer2:
            sdt = ml_dtypes.float8_e4m3 if FP8_L2_STREAM else None
            m = {"stream": plan.build_stream(k, table, dtype=sdt)}
        else:
            m = {"stream0": plan.build_stream_c(k, table, 0),
                 "stream1": plan.build_stream_c(
                     k, table, 1, dtype=ml_dtypes.float8_e4m3)}
        m["wsl"] = plan.wslab[k]
        m["bias"] = biasr
        if layer2:
            wnr = np.zeros((128, FCH * cfg.OUT), BF16)
            for c in range(FCH):
                wnr[:, c * cfg.OUT:(c + 1) * cfg.OUT] = \
                    Wn[c * 128:(c + 1) * 128, :].astype(BF16)
            m["wnext"] = wnr
        maps.append(m)
    return maps


# ---------------------------------------------------------------- driver

def kernel_run(inputs, cfg=None, trace=False):
    from concourse.bass_utils import run_bass_kernel_spmd

    cfg = cfg or Cfg()
    x = np.asarray(inputs["x"], np.float32)
    plan = Plan(cfg, np.asarray(inputs["edge_index"]),
                np.asarray(inputs["edge_weight"], np.float32))
    W1 = np.asarray(inputs["W1"], np.float32)
    b1 = np.asarray(inputs["b1"], np.float32)
    W2 = np.asarray(inputs["W2"], np.float32)
    b2 = np.asarray(inputs["b2"], np.float32)
    Wp = np.asarray(inputs["Wp"], np.float32)
    bp = np.asarray(inputs["bp"], np.float32)

    results = []

    def run(build, maps, outname):
        nc = build()
        r = run_bass_kernel_spmd(nc, maps, list(range(cfg.NCORES)),
                                 trace=trace)
        results.append(r)
        return r.results

    def as_bf16(a):
        a = np.asarray(a)
        return a if a.dtype == BF16 else a.view(BF16)

    def unpack(a, F):
        # [128, NT*F] partition-major -> [NP, F] row-major
        return np.ascontiguousarray(
            a.reshape(128, cfg.NTILES, F).transpose(1, 0, 2)
            .reshape(cfg.NP, F))

    def unpack_T(a, F):
        # [F, NT*128] transposed tiles -> [NP, F] row-major
        return np.ascontiguousarray(
            a.reshape(F, cfg.NTILES, 128).transpose(1, 2, 0)
            .reshape(cfg.NP, F))

    # fold the post-projection into layer 2: A(relu1@W2)@Wp = A(relu1@(W2@Wp))
    W2p = (W2 @ Wp).astype(np.float32)
    bpp = (b2 @ Wp + bp).astype(np.float32)

    r1 = run(lambda: _build_l1(cfg), _pack_l1_inputs(cfg, plan, x, W1), "h1")
    T1 = np.concatenate([unpack(as_bf16(r["h1"]), cfg.HID) for r in r1],
                        axis=0)

    r2 = run(lambda: _build_mp(cfg, plan, True),
             _pack_mp_inputs(cfg, plan, T1, W2p, b1, True), "out")
    T2 = np.concatenate([unpack_T(as_bf16(r["out"]), cfg.OUT) for r in r2],
                        axis=0)

    r3 = run(lambda: _build_mp(cfg, plan, False),
             _pack_mp_inputs(cfg, plan, T2, None, bpp, False), "out")

    y = np.empty((cfg.N, cfg.OUT), np.float32)
    for k in range(cfg.NCORES):
        shard = unpack_T(as_bf16(r3[k]["out"]), cfg.OUT).astype(np.float32)
        y[plan.nodes[k]] = shard[:cfg.ND]
    return y, results


def kernel(**inputs):
    y, _ = kernel_run(inputs)
    return y
